# revision 1
# baseline (speedup 1.0000x reference)
"""Trainium2 Bass kernel for BertLinearSelfAttention (linear attention).

Reference computation (per batch b, head h):
    q,k,v = X @ W{q,k,v} + b{q,k,v}            # [S, D] -> heads of 64
    qf, kf = elu(q)+1, elu(k)+1                # = min(exp(x),1) + max(x,0)
    kv[d,e]  = sum_s kf[s,d] v[s,e]            # [64, 64]
    ksum[d]  = sum_s kf[s,d]
    out[s,e] = (sum_d qf[s,d] kv[d,e]) / (sum_d qf[s,d] ksum[d])

Sharding: 8 cores = (4 batches) x (2 head-groups of 8 heads / 512 proj cols).
X is fed pre-transposed ([D, S], contraction dim on partitions) and weights in
their natural [D, CG] layout, both declared fp32r so they stream straight from
HBM into the PE with no on-device transpose or rounding pass.

All matmuls run in fp32r (single "HIGH" pass, full PE rate, ~2^-13 rounding).
Pass A: k/v projections + feature maps + kv/ksum accumulation per 512-token
chunk. Pass B: q^T projection + block-diagonal numerator/denominator matmuls
+ divide. The PE stream is software-pipelined: consumers of DVE/ACT results
(kv of chunk i, num of chunk j) are emitted one chunk late so the PE never
stalls on the elementwise chains (keeps the HAM clock at 2.4 GHz).
"""

import os
import sys

import numpy as np

_REPO = "/opt/trn_rl_repo"
if os.path.isdir(_REPO) and _REPO not in sys.path:
    sys.path.insert(0, _REPO)

B, S, D, H, HD = 4, 4096, 1024, 16, 64
NCORES = 8
CG = 512            # projection columns per core (8 heads)
NH = CG // HD       # 8 heads per core
HE = HD + 2         # head cols incl ksum column + even-pad (fp32r needs even N)
CHUNK = 512         # tokens per chunk
NSUB = CHUNK // 128     # 4 token sub-tiles per chunk
NCHUNK = S // CHUNK     # 8 chunks
NKT = D // 128          # 8 contraction tiles
P = 128

_CACHED_NC = None


def _build():
    import concourse.tile as tile
    from concourse import bacc, mybir
    from contextlib import ExitStack

    F32 = mybir.dt.float32
    F32R = mybir.dt.float32r
    Alu = mybir.AluOpType
    Act = mybir.ActivationFunctionType

    nc = bacc.Bacc("TRN2", target_bir_lowering=False, debug=False,
                   num_devices=NCORES)

    xt_d = nc.dram_tensor("xt", [D, S], F32R, kind="ExternalInput").ap()
    w_d = {
        "q": nc.dram_tensor("wq", [D, CG], F32R, kind="ExternalInput").ap(),
        "k": nc.dram_tensor("wk", [D, CG], F32R, kind="ExternalInput").ap(),
        "v": nc.dram_tensor("wv", [D, CG], F32R, kind="ExternalInput").ap(),
    }
    bq_d = nc.dram_tensor("bq", [CG], F32, kind="ExternalInput").ap()
    bk_d = nc.dram_tensor("bk", [1, CG], F32R, kind="ExternalInput").ap()
    bv_d = nc.dram_tensor("bv", [1, CG], F32, kind="ExternalInput").ap()
    ones_d = nc.dram_tensor("onesr", [1, P], F32R, kind="ExternalInput").ap()
    out_d = nc.dram_tensor("out", [S, CG], F32, kind="ExternalOutput").ap()

    with tile.TileContext(nc) as tc:
        with ExitStack() as ctx:
            const = ctx.enter_context(tc.tile_pool(name="const", bufs=1))
            wpool = ctx.enter_context(tc.tile_pool(name="wpool", bufs=1))
            xtpool = ctx.enter_context(tc.tile_pool(name="xtpool", bufs=14))

            def load_xt(ci):
                tok0 = ci * CHUNK
                xt = []
                for kt in range(NKT):
                    t = xtpool.tile([P, CHUNK], F32R, tag="xt", name="xt")
                    nc.sync.dma_start(
                        t[:], xt_d[kt * P:(kt + 1) * P, tok0:tok0 + CHUNK])
                    xt.append(t)
                return xt

            # queue the first chunk's X^T ahead of all setup DMAs
            xt0 = load_xt(0)

            # ---- constants / weights (one-time) ----
            ones_r = const.tile([1, P], F32R, tag="onesr")
            nc.sync.dma_start(ones_r[:], ones_d[:])
            bk_r = const.tile([1, CG], F32R, tag="bkr")
            nc.sync.dma_start(bk_r[:], bk_d[:])

            # q bias per-partition: bq_sb[:, ct] = bq[ct*128:(ct+1)*128]
            bq_sb = const.tile([P, CG // P], F32, tag="bqsb")
            nc.sync.dma_start(bq_sb[:], bq_d.rearrange("(c p) -> p c", p=P))

            # tail columns for V': [1.0 (ksum), 0.0 (pad)] per head
            ones_col = const.tile([P, NH * 2], F32, tag="onescol")
            nc.vector.memset(ones_col[:], 0.0)
            nc.vector.memset(
                ones_col[:].rearrange("p (h e) -> p h e", e=2)[:, :, 0:1], 1.0)

            # v bias replicated to all partitions (added during V' evict)
            bv32 = const.tile([1, CG], F32, tag="bv32")
            nc.sync.dma_start(bv32[:], bv_d[:])
            bv_rep = const.tile([P, CG], F32, tag="bvrep")
            nc.gpsimd.partition_broadcast(bv_rep[:], bv32[:])

            # weights, fp32r straight from DRAM (gpsimd queue; keeps the sync
            # queue free for the first X^T tiles)
            w_r = {}
            for nm in ("k", "v", "q"):
                w_r[nm] = wpool.tile([P, NKT * CG], F32R, tag=f"w{nm}r",
                                     name=f"w{nm}r")
                for kt in range(NKT):
                    nc.gpsimd.dma_start(w_r[nm][:, kt * CG:(kt + 1) * CG],
                                        w_d[nm][kt * P:(kt + 1) * P, :])

            # kv + ksum accumulator (SBUF side, f32; feeds the kvblocks)
            kv_sb = wpool.tile([HD, NH * HE], F32, tag="kvsb")
            nc.vector.memset(kv_sb[:], 0.0)
            # block-diagonal kv per c-tile: rows 0:64 = head 2ct (cols 0:HE),
            # rows 64:128 = head 2ct+1 (cols HE:2HE); zeros elsewhere.
            # Lets the num matmul use the full K=128 array per c-tile.
            kvblocks = [wpool.tile([P, 2 * HE], F32R, tag=f"kvb{i}",
                                   name=f"kvb{i}") for i in range(CG // P)]

            kfpool = ctx.enter_context(tc.tile_pool(name="kfpool", bufs=9))
            vppool = ctx.enter_context(tc.tile_pool(name="vppool", bufs=9))
            qftpool = ctx.enter_context(tc.tile_pool(name="qftpool", bufs=9))
            tmp = ctx.enter_context(tc.tile_pool(name="tmp", bufs=8))
            outpool = ctx.enter_context(tc.tile_pool(name="outp", bufs=6))
            rcpool = ctx.enter_context(tc.tile_pool(name="rcp", bufs=16))
            pps = ctx.enter_context(
                tc.tile_pool(name="pps", bufs=4, space="PSUM"))
            sps = ctx.enter_context(
                tc.tile_pool(name="sps", bufs=4, space="PSUM"))

            kf_c = {}   # chunk -> list of kf tiles (per sub)
            vp_c = {}
            qft_c = {}  # chunk -> list of q_feat^T tiles (per ctile)

            def a_chunk(ci, xt=None):
                """Pass A for chunk ci: k/v projections + feature maps."""
                if xt is None:
                    xt = load_xt(ci)
                kfs, vps = [], []
                for nm in ("k", "v"):
                    for sub in range(NSUB):
                        ps = pps.tile([P, CG], F32, tag="pps", name="pps")
                        for kt in range(NKT):
                            nc.tensor.matmul(
                                ps[:],
                                xt[kt][:, sub * P:(sub + 1) * P],
                                w_r[nm][:, kt * CG:(kt + 1) * CG],
                                start=(kt == 0),
                                stop=(nm == "v" and kt == NKT - 1))
                        if nm == "k":
                            # + bias via K=1 matmul
                            nc.tensor.matmul(ps[:], ones_r[:], bk_r[:],
                                             start=False, stop=True)
                            # kf = min(exp(k),1) + max(k,0)   (fp32r out)
                            e = tmp.tile([P, CG], F32, tag="t", name="t_e")
                            nc.scalar.activation(e[:], ps[:], Act.Exp)
                            m = tmp.tile([P, CG], F32, tag="t", name="t_m")
                            nc.vector.tensor_scalar(
                                m[:], e[:], 1.0, None, Alu.min)
                            r = tmp.tile([P, CG], F32, tag="t", name="t_r")
                            nc.vector.tensor_scalar(
                                r[:], ps[:], 0.0, None, Alu.max)
                            kf = kfpool.tile([P, CG], F32R, tag="kf",
                                             name="kf")
                            nc.vector.tensor_tensor(kf[:], m[:], r[:], Alu.add)
                            kfs.append(kf)
                        else:
                            # V' = [v + bv | 1 | 0] per head (fp32r out)
                            vp = vppool.tile([P, NH * HE], F32R, tag="vp",
                                             name="vp")
                            nc.vector.tensor_tensor(
                                vp[:].rearrange(
                                    "p (h e) -> p h e", e=HE)[:, :, :HD],
                                ps[:].rearrange("p (h e) -> p h e", e=HD),
                                bv_rep[:].rearrange(
                                    "p (h e) -> p h e", e=HD),
                                Alu.add)
                            nc.vector.tensor_copy(
                                vp[:].rearrange(
                                    "p (h e) -> p h e", e=HE)[:, :, HD:],
                                ones_col[:].rearrange(
                                    "p (h e) -> p h e", e=2))
                            vps.append(vp)
                kf_c[ci] = kfs
                vp_c[ci] = vps

            def a_kv(ci):
                """kv/ksum accumulation for chunk ci (one bank per head)."""
                kfs, vps = kf_c.pop(ci), vp_c.pop(ci)
                for h in range(NH):
                    kvt = sps.tile([HD, HE], F32, tag="sps", name="kvt")
                    for sub in range(NSUB):
                        nc.tensor.matmul(
                            kvt[:],
                            kfs[sub][:, h * HD:(h + 1) * HD],
                            vps[sub][:, h * HE:(h + 1) * HE],
                            start=(sub == 0), stop=(sub == NSUB - 1))
                    acc = kv_sb[:, h * HE:(h + 1) * HE]
                    nc.vector.tensor_tensor(acc, acc, kvt[:], Alu.add)

            def b_chunk(cj):
                """Pass B for chunk cj: q^T projection + feature map."""
                xtb = load_xt(cj)
                qft = []
                for ct in range(CG // P):
                    ps = pps.tile([P, CHUNK], F32, tag="pps", name="qps")
                    for kt in range(NKT):
                        nc.tensor.matmul(
                            ps[:],
                            w_r["q"][:, kt * CG + ct * P: kt * CG + (ct + 1) * P],
                            xtb[kt][:],
                            start=(kt == 0), stop=(kt == NKT - 1))
                    bcol = bq_sb[:, ct:ct + 1]
                    e = tmp.tile([P, CHUNK], F32, tag="t", name="t_qe")
                    nc.scalar.activation(e[:], ps[:], Act.Exp, bias=bcol)
                    m = tmp.tile([P, CHUNK], F32, tag="t", name="t_qm")
                    nc.vector.tensor_scalar(m[:], e[:], 1.0, None, Alu.min)
                    r = tmp.tile([P, CHUNK], F32, tag="t", name="t_qr")
                    nc.vector.tensor_scalar(
                        r[:], ps[:], bcol, 0.0, Alu.add, Alu.max)
                    qf = qftpool.tile([P, CHUNK], F32R, tag="qft", name="qft")
                    nc.vector.tensor_tensor(qf[:], m[:], r[:], Alu.add)
                    qft.append(qf)
                qft_c[cj] = qft

            def b_num(cj):
                """num/den matmuls + divide + store for chunk cj."""
                tok0 = cj * CHUNK
                qft = qft_c.pop(cj)
                outs = [outpool.tile([P, CG], F32, tag="out", name=f"osb{i}")
                        for i in range(NSUB)]
                for sub in range(NSUB):
                    for ct in range(CG // P):
                        # [num|den|pad] for heads (2ct, 2ct+1) in one matmul
                        pn = sps.tile([P, 2 * HE], F32, tag="sps", name="pn")
                        nc.tensor.matmul(
                            pn[:],
                            qft[ct][:, sub * P:(sub + 1) * P],
                            kvblocks[ct][:],
                            start=True, stop=True)
                        rc = rcpool.tile([P, 2], F32, tag="rc", name="rc")
                        nc.vector.reciprocal(
                            rc[:].rearrange("p (h e) -> p h e", e=1),
                            pn[:].rearrange(
                                "p (h e) -> p h e", e=HE)[:, :, HD:HD + 1])
                        # out = num * (1/den), per-partition scale on ACT
                        for half in range(2):
                            nc.scalar.mul(
                                outs[sub][:, (2 * ct + half) * HD:
                                          (2 * ct + half + 1) * HD],
                                pn[:, half * HE:half * HE + HD],
                                rc[:, half:half + 1])
                for sub in range(NSUB):
                    nc.sync.dma_start(
                        out_d[tok0 + sub * P: tok0 + (sub + 1) * P, :],
                        outs[sub][:])

            # ---- software-pipelined stream ----
            for ci in range(NCHUNK):
                a_chunk(ci, xt0 if ci == 0 else None)
                if ci >= 1:
                    a_kv(ci - 1)
            b_chunk(0)          # q^T needs no kv; bridges the A->B gap
            a_kv(NCHUNK - 1)
            # kv complete -> build block-diagonal fp32r kvblocks
            for ct in range(CG // P):
                kstg = outpool.tile([P, 2 * HE], F32, tag="out", name="kstg")
                nc.vector.memset(kstg[:], 0.0)
                nc.vector.tensor_copy(
                    kstg[0:HD, 0:HE],
                    kv_sb[:, (2 * ct) * HE:(2 * ct + 1) * HE])
                nc.vector.tensor_copy(
                    kstg[HD:P, HE:2 * HE],
                    kv_sb[:, (2 * ct + 1) * HE:(2 * ct + 2) * HE])
                nc.vector.tensor_copy(kvblocks[ct][:], kstg[:])
            for cj in range(1, NCHUNK):
                b_chunk(cj)
                b_num(cj - 1)
            b_num(NCHUNK - 1)

    nc.compile()
    return nc


def _get_nc():
    global _CACHED_NC
    if _CACHED_NC is None:
        _CACHED_NC = _build()
    return _CACHED_NC


def _make_in_maps(hidden_states, Wq, bq, Wk, bk, Wv, bv):
    hs = np.asarray(hidden_states, np.float32)
    ones = np.ones((1, P), np.float32)
    arrs = {"wq": np.asarray(Wq, np.float32), "wk": np.asarray(Wk, np.float32),
            "wv": np.asarray(Wv, np.float32), "bq": np.asarray(bq, np.float32),
            "bk": np.asarray(bk, np.float32), "bv": np.asarray(bv, np.float32)}
    xts = [np.ascontiguousarray(hs[b].T) for b in range(B)]
    in_maps = []
    for c in range(NCORES):
        b, g = divmod(c, 2)
        sl = slice(g * CG, (g + 1) * CG)
        in_maps.append({
            "xt": xts[b],
            "wq": np.ascontiguousarray(arrs["wq"][:, sl]),
            "wk": np.ascontiguousarray(arrs["wk"][:, sl]),
            "wv": np.ascontiguousarray(arrs["wv"][:, sl]),
            "bq": np.ascontiguousarray(arrs["bq"][sl]),
            "bk": np.ascontiguousarray(arrs["bk"][sl]).reshape(1, CG),
            "bv": np.ascontiguousarray(arrs["bv"][sl]).reshape(1, CG),
            "onesr": ones,
        })
    return in_maps


def _run(in_maps, **kwargs):
    from concourse.bass_utils import run_bass_kernel_spmd
    nc = _get_nc()
    return run_bass_kernel_spmd(nc, in_maps, core_ids=list(range(NCORES)),
                                **kwargs)


def _assemble(results):
    out = np.empty((B, S, D), np.float32)
    for c in range(NCORES):
        b, g = divmod(c, 2)
        out[b, :, g * CG:(g + 1) * CG] = results[c]["out"]
    return out


def kernel(hidden_states, Wq, bq, Wk, bk, Wv, bv):
    in_maps = _make_in_maps(hidden_states, Wq, bq, Wk, bk, Wv, bv)
    res = _run(in_maps)
    return _assemble(res.results)



# revision 11
# speedup vs baseline: 1.3380x; 1.3380x over previous
"""Trainium2 Bass kernel for BertLinearSelfAttention (linear attention).

Reference computation (per batch b, head h):
    q,k,v = X @ W{q,k,v} + b{q,k,v}            # [S, D] -> heads of 64
    qf, kf = elu(q)+1, elu(k)+1                # = min(exp(x),1) + max(x,0)
    kv[d,e]  = sum_s kf[s,d] v[s,e]            # [64, 64]
    ksum[d]  = sum_s kf[s,d]
    out[s,e] = (sum_d qf[s,d] kv[d,e]) / (sum_d qf[s,d] ksum[d])

Sharding: 8 cores = (4 batches) x (2 head-groups of 8 heads / 512 proj cols).
All matmul operands are bf16 (converted host-side), which keeps every matmul
at the PE's 1 row/cycle stream rate: fp32 operands are SBUF-read-bandwidth
bound (~0.92 ns/row measured vs 0.42 compute), bf16 halves the traffic.

Pass A (per 512-token chunk): k/v projections (tokens on partitions) +
feature maps on DVE/ACT + per-head kv/ksum accumulated directly in PSUM
across all chunks (ones-column in V' produces ksum).
Pass B (per chunk): q^T projection (cols on partitions), then per head-pair
block-diagonal den/num matmuls with N=512 moving tokens:
    den^T[p,s] = sum_k ksumrep[k,p] qf^T[k,s]   (ksum replicated across the
                 64 e-columns of its head, so the PE broadcasts den for free)
    num^T[e,s] = sum_d kv[d,e] qf^T[d,s]
Divide runs on DVE as the PSUM evict (recip + mult), output is stored
transposed [cols, tokens] in bf16 and re-transposed/upcast on the host.
"""

import os
import sys

import numpy as np

_REPO = "/opt/trn_rl_repo"
if os.path.isdir(_REPO) and _REPO not in sys.path:
    sys.path.insert(0, _REPO)

B, S, D, H, HD = 4, 4096, 1024, 16, 64
NCORES = 8
CG = 512            # projection columns per core (8 heads)
NH = CG // HD       # 8 heads per core
NCT = CG // 128     # 4 head-pair column tiles
HE = HD + 2         # vp cols per head: 64 v + 1 ones (ksum) + 1 pad
CHUNK = 512         # tokens per chunk
NSUB = CHUNK // 128     # 4 token sub-tiles per chunk
NCHUNK = S // CHUNK     # 8 chunks
NKT = D // 128          # 8 contraction tiles
P = 128

_CACHED_NC = None


def _build():
    import concourse.tile as tile
    from concourse import bacc, mybir
    from contextlib import ExitStack

    F32 = mybir.dt.float32
    BF16 = mybir.dt.bfloat16
    Alu = mybir.AluOpType
    Act = mybir.ActivationFunctionType

    nc = bacc.Bacc("TRN2", target_bir_lowering=False, debug=False,
                   num_devices=NCORES)

    xt_d = nc.dram_tensor("xt", [D, S], BF16, kind="ExternalInput").ap()
    w_d = {
        "q": nc.dram_tensor("wq", [D, CG], BF16, kind="ExternalInput").ap(),
        "k": nc.dram_tensor("wk", [D, CG], BF16, kind="ExternalInput").ap(),
        "v": nc.dram_tensor("wv", [D, CG], BF16, kind="ExternalInput").ap(),
    }
    bq_d = nc.dram_tensor("bq", [P, NCT], F32, kind="ExternalInput").ap()
    bk_d = nc.dram_tensor("bk", [1, CG], F32, kind="ExternalInput").ap()
    bv_d = nc.dram_tensor("bv", [1, CG], F32, kind="ExternalInput").ap()
    out_d = nc.dram_tensor("out", [S, CG], BF16, kind="ExternalOutput").ap()

    with tile.TileContext(nc) as tc:
        with ExitStack() as ctx:
            const = ctx.enter_context(tc.tile_pool(name="const", bufs=1))
            wpool = ctx.enter_context(tc.tile_pool(name="wpool", bufs=1))
            xtpool = ctx.enter_context(tc.tile_pool(name="xtpool", bufs=24))
            kfpool = ctx.enter_context(tc.tile_pool(name="kfpool", bufs=8))
            vppool = ctx.enter_context(tc.tile_pool(name="vppool", bufs=8))
            qftpool = ctx.enter_context(tc.tile_pool(name="qftpool", bufs=8))
            tmp = ctx.enter_context(tc.tile_pool(name="tmp", bufs=10))
            outpool = ctx.enter_context(tc.tile_pool(name="outp", bufs=6))
            recpool = ctx.enter_context(tc.tile_pool(name="recp", bufs=8))
            kvbpool = ctx.enter_context(tc.tile_pool(name="kvbp", bufs=1))
            pps = ctx.enter_context(
                tc.tile_pool(name="pps", bufs=2, space="PSUM"))
            kvps = ctx.enter_context(
                tc.tile_pool(name="kvps", bufs=1, space="PSUM"))
            dnps = ctx.enter_context(
                tc.tile_pool(name="dnps", bufs=2, space="PSUM"))

            def load_xt(ci):
                tok0 = ci * CHUNK
                xt = []
                for kt in range(NKT):
                    t = xtpool.tile([P, CHUNK], BF16, tag="xt", name="xt")
                    nc.sync.dma_start(
                        t[:], xt_d[kt * P:(kt + 1) * P, tok0:tok0 + CHUNK])
                    xt.append(t)
                return xt

            # queue the first chunk's X^T ahead of all setup DMAs
            xt0 = load_xt(0)

            # ---- weights (per-kt tiles so the first matmul only waits on
            # its own 128KB slice), k/v interleaved first, q later ----
            w_t = {"q": [], "k": [], "v": []}
            for kt in range(NKT):
                for nm in ("k", "v"):
                    t = wpool.tile([P, CG], BF16, tag=f"w{nm}{kt}",
                                   name=f"w{nm}{kt}")
                    nc.gpsimd.dma_start(t[:], w_d[nm][kt * P:(kt + 1) * P, :])
                    w_t[nm].append(t)
            for kt in range(NKT):
                t = wpool.tile([P, CG], BF16, tag=f"wq{kt}", name=f"wq{kt}")
                nc.gpsimd.dma_start(t[:], w_d["q"][kt * P:(kt + 1) * P, :])
                w_t["q"].append(t)

            # ---- constants ----
            bk_sb = const.tile([1, CG], F32, tag="bk", name="bk_sb")
            nc.gpsimd.dma_start(bk_sb[:], bk_d[:])
            bv_sb = const.tile([1, CG], F32, tag="bv", name="bv_sb")
            nc.gpsimd.dma_start(bv_sb[:], bv_d[:])
            bq_sb = const.tile([P, NCT], F32, tag="bq", name="bq_sb")
            nc.gpsimd.dma_start(bq_sb[:], bq_d[:])
            bk_rep = const.tile([P, CG], F32, tag="bkrep", name="bk_rep")
            nc.gpsimd.partition_broadcast(bk_rep[:], bk_sb[:])
            bv_rep = const.tile([P, CG], F32, tag="bvrep", name="bv_rep")
            nc.gpsimd.partition_broadcast(bv_rep[:], bv_sb[:])
            # tail columns for V': [1.0 (ksum), 0.0 (pad)] per head
            ones_tail = const.tile([P, NH * 2], BF16, tag="otail",
                                   name="ones_tail")
            nc.vector.memset(ones_tail[:], 0.0)
            nc.vector.memset(
                ones_tail[:].rearrange("p (h e) -> p h e", e=2)[:, :, 0:1],
                1.0)

            # kv/ksum accumulators: bank i holds ct=2i (cols 0:132) and
            # ct=2i+1 (cols 132:264); within a ct: even head on partitions
            # 0:64 cols 0:66, odd head on partitions 64:128 cols 66:132.
            # NOTE: matmul start=True zeroes the full bank width for the
            # partitions it writes, so concurrent accumulation groups in one
            # bank must NOT use start; memset once and accumulate throughout.
            kvacc = [kvps.tile([P, 4 * HE], F32, tag=f"kvacc{i}",
                               name=f"kvacc{i}") for i in range(2)]
            for i in range(2):
                nc.vector.memset(kvacc[i][:], 0.0)

            kf_c = {}
            vp_c = {}
            qft_c = {}

            def a_chunk(ci, xt):
                kfs, vps = [], []
                for sub in range(NSUB):
                    sl = slice(sub * P, (sub + 1) * P)
                    kps = pps.tile([P, CG], F32, tag="pps", name="kps")
                    for kt in range(NKT):
                        nc.tensor.matmul(
                            kps[:], xt[kt][:, sl], w_t["k"][kt][:],
                            start=(kt == 0), stop=(kt == NKT - 1))
                    vps_ = pps.tile([P, CG], F32, tag="pps", name="vps")
                    for kt in range(NKT):
                        nc.tensor.matmul(
                            vps_[:], xt[kt][:, sl], w_t["v"][kt][:],
                            start=(kt == 0), stop=(kt == NKT - 1))
                    # k feature map: kf = min(exp(k+bk),1) + max(k+bk,0)
                    kb = tmp.tile([P, CG], BF16, tag="tmp", name="kb")
                    nc.vector.tensor_tensor(kb[:], kps[:], bk_rep[:], Alu.add)
                    e = tmp.tile([P, CG], BF16, tag="tmp", name="e")
                    nc.scalar.activation(e[:], kb[:], Act.Exp)
                    r = tmp.tile([P, CG], BF16, tag="tmp", name="r")
                    nc.scalar.activation(r[:], kb[:], Act.Relu)
                    m = tmp.tile([P, CG], BF16, tag="tmp", name="m")
                    nc.vector.tensor_scalar(m[:], e[:], 1.0, None, Alu.min)
                    kf = kfpool.tile([P, CG], BF16, tag="kf", name="kf")
                    nc.vector.tensor_tensor(kf[:], m[:], r[:], Alu.add)
                    kfs.append(kf)
                    # V' = [v + bv | 1 | 0] per head
                    vp = vppool.tile([P, NH * HE], BF16, tag="vp", name="vp")
                    nc.vector.tensor_tensor(
                        vp[:].rearrange("p (h e) -> p h e", e=HE)[:, :, :HD],
                        vps_[:].rearrange("p (h e) -> p h e", e=HD),
                        bv_rep[:].rearrange("p (h e) -> p h e", e=HD),
                        Alu.add)
                    nc.vector.tensor_copy(
                        vp[:].rearrange("p (h e) -> p h e", e=HE)[:, :, HD:],
                        ones_tail[:].rearrange("p (h e) -> p h e", e=2))
                    vps.append(vp)
                kf_c[ci] = kfs
                vp_c[ci] = vps

            def a_kv(ci):
                kfs, vps = kf_c.pop(ci), vp_c.pop(ci)
                for ct in range(NCT):
                    bank = kvacc[ct // 2]
                    base = (ct % 2) * 2 * HE
                    for par in range(2):
                        h = 2 * ct + par
                        dst = bank[par * HD:(par + 1) * HD,
                                   base + par * HE:base + (par + 1) * HE]
                        for sub in range(NSUB):
                            nc.tensor.matmul(
                                dst,
                                kfs[sub][:, h * HD:(h + 1) * HD],
                                vps[sub][:, h * HE:(h + 1) * HE],
                                start=False,
                                stop=(ci == NCHUNK - 1 and sub == NSUB - 1),
                                skip_group_check=True)

            kvb = []
            ks2 = []

            def bridge():
                """kv PSUM -> bf16 block-diag matmul rhs tiles + ksum cols."""
                for ct in range(NCT):
                    bank = kvacc[ct // 2]
                    base = (ct % 2) * 2 * HE
                    b_ = kvbpool.tile([P, P], BF16, tag=f"kvb{ct}",
                                      name=f"kvb{ct}")
                    nc.vector.memset(b_[:], 0.0)
                    nc.vector.tensor_copy(
                        b_[0:HD, 0:HD], bank[0:HD, base:base + HD])
                    nc.vector.tensor_copy(
                        b_[HD:P, HD:P],
                        bank[HD:P, base + HE:base + HE + HD])
                    kvb.append(b_)
                    # ksum columns: [ksum_even | 0 ; 0 | ksum_odd]
                    s_ = kvbpool.tile([P, 2], BF16, tag=f"ks2{ct}",
                                      name=f"ks2{ct}")
                    nc.vector.memset(s_[:], 0.0)
                    nc.vector.tensor_copy(
                        s_[0:HD, 0:1], bank[0:HD, base + HD:base + HD + 1])
                    nc.vector.tensor_copy(
                        s_[HD:P, 1:2],
                        bank[HD:P, base + HE + HD:base + HE + HD + 1])
                    ks2.append(s_)

            def b_qproj(cj, xt):
                qps = []
                for ct in range(NCT):
                    ps = pps.tile([P, CHUNK], F32, tag="pps", name="qps")
                    for kt in range(NKT):
                        nc.tensor.matmul(
                            ps[:],
                            w_t["q"][kt][:, ct * P:(ct + 1) * P],
                            xt[kt][:],
                            start=(kt == 0), stop=(kt == NKT - 1))
                    qps.append(ps)
                return qps

            def b_qfm(cj, qps):
                qft = []
                for ct in range(NCT):
                    bcol = bq_sb[:, ct:ct + 1]
                    e = tmp.tile([P, CHUNK], BF16, tag="tmp", name="qe")
                    nc.scalar.activation(e[:], qps[ct][:], Act.Exp, bias=bcol)
                    r = tmp.tile([P, CHUNK], BF16, tag="tmp", name="qr")
                    nc.scalar.activation(r[:], qps[ct][:], Act.Relu, bias=bcol)
                    m = tmp.tile([P, CHUNK], BF16, tag="tmp", name="qm")
                    nc.vector.tensor_scalar(m[:], e[:], 1.0, None, Alu.min)
                    qf = qftpool.tile([P, CHUNK], BF16, tag="qft", name="qft")
                    nc.vector.tensor_tensor(qf[:], m[:], r[:], Alu.add)
                    qft.append(qf)
                qft_c[cj] = qft

            def b_dn(cj):
                """den/num matmuls for chunk cj (PE, [tokens, cols] layout).

                num_sub[s, e] accumulates per-ct 128-col blocks into one bank;
                dent[s, sub*8 + h] gets the per-head denominators. start=True
                only on each bank's first matmul (it zeroes the whole bank),
                the rest accumulate onto zeros in disjoint column ranges.
                """
                qft = qft_c[cj]
                dent = dnps.tile([P, NSUB * NH], F32, tag="dent", name="dent")
                nums = []
                for sub in range(NSUB):
                    sl = slice(sub * P, (sub + 1) * P)
                    for ct in range(NCT):
                        nc.tensor.matmul(
                            dent[:, sub * NH + 2 * ct:sub * NH + 2 * ct + 2],
                            qft[ct][:, sl], ks2[ct][:],
                            start=(sub == 0 and ct == 0),
                            stop=(sub == NSUB - 1 and ct == NCT - 1),
                            skip_group_check=True)
                    nps = dnps.tile([P, CHUNK], F32, tag="num", name="nps")
                    for ct in range(NCT):
                        nc.tensor.matmul(
                            nps[:, ct * P:(ct + 1) * P],
                            qft[ct][:, sl], kvb[ct][:],
                            start=(ct == 0), stop=(ct == NCT - 1),
                            skip_group_check=True)
                    nums.append(nps)
                return dent, nums

            def b_div(cj, dn):
                """reciprocal + broadcast-multiply evict + store (DVE + DMA)."""
                tok0 = cj * CHUNK
                qft_c.pop(cj)
                dent, nums = dn
                for sub in range(NSUB):
                    rec = recpool.tile([P, NH], F32, tag="rec", name="rec")
                    nc.vector.reciprocal(
                        rec[:], dent[:, sub * NH:(sub + 1) * NH])
                    osb = outpool.tile([P, CG], BF16, tag="out", name="osb")
                    for h in range(NH):
                        nc.vector.tensor_scalar(
                            osb[:, h * HD:(h + 1) * HD],
                            nums[sub][:, h * HD:(h + 1) * HD],
                            rec[:, h:h + 1], None, Alu.mult)
                    nc.sync.dma_start(
                        out_d[tok0 + sub * P:tok0 + (sub + 1) * P, :],
                        osb[:])

            # ---- pass A ----
            xt_cur = xt0
            for ci in range(NCHUNK):
                a_chunk(ci, xt_cur)
                xt_cur = load_xt(ci + 1) if ci + 1 < NCHUNK else None
                if ci >= 1:
                    a_kv(ci - 1)
            xtb = load_xt(0)
            a_kv(NCHUNK - 1)

            # ---- pass B ----
            qps = b_qproj(0, xtb)
            bridge()
            b_qfm(0, qps)
            xtb = load_xt(1)
            dn_prev = None
            for cj in range(1, NCHUNK):
                qps = b_qproj(cj, xtb)
                xtb = load_xt(cj + 1) if cj + 1 < NCHUNK else None
                dn_prev = b_dn(cj - 1)
                b_div(cj - 1, dn_prev)
                b_qfm(cj, qps)
            dn_prev = b_dn(NCHUNK - 1)
            b_div(NCHUNK - 1, dn_prev)

    nc.compile()
    return nc


def _get_nc():
    global _CACHED_NC
    if _CACHED_NC is None:
        _CACHED_NC = _build()
    return _CACHED_NC


def _make_in_maps(hidden_states, Wq, bq, Wk, bk, Wv, bv):
    import ml_dtypes

    BF = ml_dtypes.bfloat16
    hs = np.asarray(hidden_states, np.float32)
    wq = np.asarray(Wq, np.float32)
    wk = np.asarray(Wk, np.float32)
    wv = np.asarray(Wv, np.float32)
    bq_ = np.asarray(bq, np.float32)
    bk_ = np.asarray(bk, np.float32)
    bv_ = np.asarray(bv, np.float32)
    xts = [np.ascontiguousarray(hs[b].T).astype(BF) for b in range(B)]
    in_maps = []
    for c in range(NCORES):
        b, g = divmod(c, 2)
        sl = slice(g * CG, (g + 1) * CG)
        in_maps.append({
            "xt": xts[b],
            "wq": wq[:, sl].astype(BF),
            "wk": wk[:, sl].astype(BF),
            "wv": wv[:, sl].astype(BF),
            "bq": np.ascontiguousarray(bq_[sl].reshape(NCT, P).T),
            "bk": np.ascontiguousarray(bk_[sl]).reshape(1, CG),
            "bv": np.ascontiguousarray(bv_[sl]).reshape(1, CG),
        })
    return in_maps


def _run(in_maps, **kwargs):
    from concourse.bass_utils import run_bass_kernel_spmd
    nc = _get_nc()
    return run_bass_kernel_spmd(nc, in_maps, core_ids=list(range(NCORES)),
                                **kwargs)


def _assemble(results):
    out = np.empty((B, S, D), np.float32)
    for c in range(NCORES):
        b, g = divmod(c, 2)
        out[b, :, g * CG:(g + 1) * CG] = results[c]["out"].astype(np.float32)
    return out


def kernel(hidden_states, Wq, bq, Wk, bk, Wv, bv):
    in_maps = _make_in_maps(hidden_states, Wq, bq, Wk, bk, Wv, bv)
    res = _run(in_maps)
    return _assemble(res.results)
